# revision 1
# baseline (speedup 1.0000x reference)
"""Trainium2 Bass kernel for nn_EnetGnn (GNN message passing with knn graph).

Math (per batch b, 3 GNN iterations):
  x = positions (proj_3d for it 0, else h); knn_16(x) per row.
  z = 2-layer PReLU MLP of h (per node);  m_i = mean of z over i's 16 nn.
  h = relu([h, m] @ q_W.T + q_b)

Reformulation (no per-row index gathers on device):
  S[i,j] = 2 x_i.x_j - |x_j|^2 ranks identically to -D2 per row.
  v16 = exact 16th-largest of S_i via a chunked max8 sweep (Vector engine).
  A fused 5-row matmul computes S.T - v16 with j on partitions; a Sign map
  (Scalar engine) and a strict >0 map (Vector engine) feed mask matmuls
  against z (plus a ones column for counts):
     B = {S > v16}:  Sum_B z = P_gt,        n_B = C_gt
     A = {S >= v16}: Sum_A z = P_sg + g - P_gt,  n_A = C_sg + HW - C_gt
  Tie-exact mean: m = [Sum_B + (16-n_B)*(Sum_A-Sum_B)/(n_A-n_B)] / 16.
  (Boundary ties are duplicate points with identical z, matching the
  reference's lowest-index tie-break exactly.)

Sharding: core c handles batch c//2, row-half c%2 (4608 rows).  Core pairs
exchange updated h halves with a 2-core AllGather between iterations.
"""

import os
import sys
import numpy as np

for _p in ("/opt/trn_rl_repo", "/root/.axon_site/_ro/trn_rl_repo"):
    if os.path.isdir(_p) and _p not in sys.path:
        sys.path.append(_p)

import concourse.bass as bass
import concourse.bacc as bacc
import concourse.mybir as mybir
from concourse import tile
from concourse.bass_utils import run_bass_kernel_spmd

F32 = mybir.dt.float32
U32 = mybir.dt.uint32
AF = mybir.ActivationFunctionType
ALU = mybir.AluOpType
AX = mybir.AxisListType

N, C, H, W = 4, 3, 96, 96
HW = H * W            # 9216
RP = HW // 2          # 4608 rows per core
NT = RP // 128        # 36 row tiles
JT = HW // 128        # 72 col j-tiles
NCH = HW // 512       # 18
ITERS = 3
SWEEP = 512           # sweep chunk width
NSW = HW // SWEEP     # 18 chunks -> cand 144 wide
NEG_BIG = -3.0e38

IC_W = [1024, 1024, 1024, 1024, 512]
IC_OFF = [0, 1024, 2048, 3072, 4096]


def _build_program():
    nc = bacc.Bacc(None, target_bir_lowering=False, num_devices=8)

    x0 = nc.declare_dram_parameter("x0", [3, HW], F32, isOutput=False)
    pj = nc.declare_dram_parameter("pj", [3, HW], F32, isOutput=False)
    h0o = nc.declare_dram_parameter("h0own", [3, RP], F32, isOutput=False)
    ident = nc.declare_dram_parameter("ident", [128, 128], F32, isOutput=False)
    m01 = nc.declare_dram_parameter("m01", [3, 2], F32, isOutput=False)
    gw = nc.declare_dram_parameter("gw", [3, 6], F32, isOutput=False)   # g_W[l].T pair
    gb = nc.declare_dram_parameter("gb", [3, 2], F32, isOutput=False)
    ga = nc.declare_dram_parameter("ga", [3, 2], F32, isOutput=False)
    qw = nc.declare_dram_parameter("qw", [6, 3], F32, isOutput=False)   # q_W.T
    qb = nc.declare_dram_parameter("qb", [3, 1], F32, isOutput=False)
    ones3 = nc.declare_dram_parameter("ones3", [3, 1], F32, isOutput=False)
    onesrow = nc.declare_dram_parameter("onesrow", [1, HW], F32, isOutput=False)
    zrow = nc.declare_dram_parameter("zrow", [1, RP], F32, isOutput=False)
    out = nc.declare_dram_parameter("out", [3, RP], F32, isOutput=True)
    dbg = nc.declare_dram_parameter("dbg", [128, 64], F32, isOutput=True)
    dbg2 = nc.declare_dram_parameter("dbg2", [8, 512], F32, isOutput=True)
    n_iters = int(os.environ.get("KB_ITERS", str(ITERS)))
    skip_row = bool(int(os.environ.get("KB_SKIP_ROW", "0")))
    skip_col = bool(int(os.environ.get("KB_SKIP_COL", "0")))
    skip_mlp = bool(int(os.environ.get("KB_SKIP_MLP", "0")))

    with tile.TileContext(nc, num_cores=8) as tc:
        with (
            tc.tile_pool(name="dram", bufs=2, space="DRAM") as dram,
            tc.tile_pool(name="big1", bufs=1) as big1,
            tc.tile_pool(name="spc", bufs=3) as spc,
            tc.tile_pool(name="msk", bufs=2) as msk,
            tc.tile_pool(name="sm", bufs=2) as sm,
            tc.tile_pool(name="vp", bufs=4) as vp,
            tc.tile_pool(name="chk", bufs=2) as chk,
            tc.tile_pool(name="epi", bufs=1) as epi,
            tc.tile_pool(name="psbig", bufs=2, space="PSUM") as psbig,
            tc.tile_pool(name="psmq", bufs=2, space="PSUM") as psmq,
        ):
            A5 = big1.tile([5, HW], F32, tag="A5")    # 2x0,2x1,2x2,-d,1
            hT = big1.tile([3, HW], F32, tag="h")
            B5 = big1.tile([5, RP], F32, tag="B5")    # own x,1 | row4: 0 in row phase, then -theta_lo
            B5H = big1.tile([5, RP], F32, tag="B5H")  # own x,1 | row4: -theta_hi
            zcm = big1.tile([128, JT * 4], F32, tag="zcm")
            identt = big1.tile([128, 128], F32, tag="ident")
            T36 = big1.tile([128, NT], F32, tag="T36")
            cw = big1.tile([3, 16], F32, tag="cw")    # gw(0:6) gb(6:8) ga(8:10) m01(10:12) qb(12)
            qwt = big1.tile([6, 3], F32, tag="qwt")
            o3 = big1.tile([3, 1], F32, tag="o3")
            o13 = big1.tile([1, 3], F32, tag="o13")
            gp = big1.tile([3, 24], F32, tag="gp")

            ccin = dram.tile([3, RP], F32, tag="ccin")
            ccout = dram.tile([6, RP], F32, tag="ccout")

            # ---- static setup ----
            nc.sync.dma_start(identt[:], ident[:])
            nc.sync.dma_start(cw[:, 0:6], gw[:])
            nc.sync.dma_start(cw[:, 6:8], gb[:])
            nc.sync.dma_start(cw[:, 8:10], ga[:])
            nc.sync.dma_start(cw[:, 10:12], m01[:])
            nc.sync.dma_start(cw[:, 12:13], qb[:])
            nc.sync.dma_start(qwt[:], qw[:])
            nc.sync.dma_start(o3[:], ones3[:])
            nc.sync.dma_start(o13[:], ones3[:])
            nc.sync.dma_start(hT[:], x0[:])
            nc.sync.dma_start(A5[4:5, :], onesrow[:])
            nc.sync.dma_start(B5[3:4, :], onesrow[0:1, 0:RP])
            nc.vector.memset(zcm[:], 1.0)

            m0 = cw[:, 10:11]
            m1 = cw[:, 11:12]

            for it in range(n_iters):
                # ---------- prep: A5 rows, B5 x rows; B5 row4 <- 0 for the row phase ----------
                nc.sync.dma_start(B5[4:5, :], zrow[:])
                for ch in range(NCH):
                    sl = slice(ch * 512, (ch + 1) * 512)
                    if it == 0:
                        xc = chk.tile([3, 512], F32, tag="xc")
                        nc.sync.dma_start(xc[:], pj[:, sl])
                        xa = xc[:]
                    else:
                        xa = hT[:, sl]
                    sq = chk.tile([3, 512], F32, tag="sq")
                    nc.gpsimd.tensor_tensor(sq[:], xa, xa, ALU.mult)
                    dps = psmq.tile([1, 512], F32, tag="mq")
                    nc.tensor.matmul(dps[:], o3[:], sq[:], start=True, stop=True)
                    nc.gpsimd.tensor_scalar(A5[0:3, sl], xa, 2.0, None, ALU.mult)
                    dnc = chk.tile([1, 512], F32, tag="dnc")
                    nc.vector.tensor_scalar(dnc[:], dps[:], -1.0, None, ALU.mult)
                    nc.sync.dma_start(A5[3:4, sl], dnc[:])

                if it == 0:
                    for ch in range(NCH // 2):
                        sl = slice(ch * 512, (ch + 1) * 512)
                        xlo = chk.tile([3, 512], F32, tag="xc")
                        xhi = chk.tile([3, 512], F32, tag="sq")
                        nc.sync.dma_start(xlo[:], pj[:, sl])
                        nc.sync.dma_start(xhi[:], pj[:, RP + ch * 512:RP + (ch + 1) * 512])
                        nc.vector.tensor_scalar(B5[0:3, sl], xlo[:], m0, None, ALU.mult)
                        nc.vector.scalar_tensor_tensor(B5[0:3, sl], xhi[:], m1,
                                                       B5[0:3, sl], ALU.mult, ALU.add)
                else:
                    nc.vector.tensor_scalar(B5[0:3, 0:RP], hT[:, 0:RP], m0, None, ALU.mult)
                    nc.vector.scalar_tensor_tensor(B5[0:3, 0:RP], hT[:, RP:], m1,
                                                   B5[0:3, 0:RP], ALU.mult, ALU.add)

                # ---------- z = MLP(h) -> zcm (node-major + ones col), g ----------
                for ch in range([] if skip_mlp else range(NCH)) if False else (range(0) if skip_mlp else range(NCH)):
                    sl = slice(ch * 512, (ch + 1) * 512)
                    z1p = psmq.tile([3, 512], F32, tag="mq")
                    nc.tensor.matmul(z1p[:], cw[:, 0:3], hT[:, sl], start=True, stop=True)
                    zf1 = chk.tile([3, 512], F32, tag="zf1")
                    nc.scalar.activation(zf1[:], z1p[:], AF.Prelu,
                                         bias=cw[:, 6:7], scale=1.0, alpha=cw[:, 8:9])
                    z2p = psmq.tile([3, 512], F32, tag="mq")
                    nc.tensor.matmul(z2p[:], cw[:, 3:6], zf1[:], start=True, stop=True)
                    zf2 = chk.tile([3, 512], F32, tag="zf2")
                    nc.scalar.activation(zf2[:], z2p[:], AF.Prelu,
                                         bias=cw[:, 7:8], scale=1.0, alpha=cw[:, 9:10],
                                         accum_out=gp[0:3, ch:ch + 1])
                    for q in range(4):
                        J = ch * 4 + q
                        tp = psmq.tile([128, 3], F32, tag="mq")
                        nc.tensor.transpose(tp[:], zf2[:, q * 128:(q + 1) * 128], identt[0:3, 0:3])
                        nc.vector.tensor_copy(zcm[:, J * 4:J * 4 + 3], tp[:])
                nc.vector.tensor_reduce(gp[0:3, 20:21], gp[0:3, 0:NCH], op=ALU.add, axis=AX.X)
                gs3 = gp[0:3, 20:21]

                # ---------- row phase: v16 per own row ----------
                for r in range(0) if skip_row else range(NT):
                    lhs = B5[0:5, r * 128:(r + 1) * 128]
                    cand = vp.tile([128, 144], F32, tag="cand")
                    for g in range(9):
                        ps = psbig.tile([128, 1024], F32, tag="ps")
                        for q in range(2):
                            jsl = slice(g * 1024 + q * 512, g * 1024 + (q + 1) * 512)
                            nc.tensor.matmul(ps[:, q * 512:(q + 1) * 512],
                                             lhs, A5[0:5, jsl], start=True, stop=True)
                        pc = spc.tile([128, 1024], F32, tag="pc")
                        if g % 3 == 2:
                            nc.vector.tensor_copy(pc[:], ps[:])
                        else:
                            nc.scalar.activation(pc[:], ps[:], AF.Copy)
                        for q in range(2):
                            c = g * 2 + q
                            nc.vector.max(cand[:, c * 8:(c + 1) * 8],
                                          pc[:, q * 512:(q + 1) * 512])
                    v8a = vp.tile([128, 8], F32, tag="v8")
                    nc.vector.max(v8a[:], cand[:])
                    nc.vector.match_replace(cand[:], v8a[:], cand[:], NEG_BIG)
                    v8b = vp.tile([128, 8], F32, tag="v8")
                    nc.vector.max(v8b[:], cand[:])
                    nc.vector.tensor_copy(T36[:, r:r + 1], v8b[:, 7:8])

                # band thresholds: -theta_lo -> B5[4], -theta_hi -> B5H[4]
                if skip_row:
                    nc.vector.memset(T36[:], 1.0)
                ALPHA = float(os.environ.get("KB_ALPHA", "2.5e-7"))
                BETA = float(os.environ.get("KB_BETA", "2.0e-6"))
                Ew = sm.tile([128, NT], F32, tag="tadE")
                Tlo = sm.tile([128, NT], F32, tag="tad")
                Thi = sm.tile([128, NT], F32, tag="tad2")
                nc.vector.tensor_scalar(Ew[:].bitcast(U32), T36[:].bitcast(U32),
                                        2147483647, None, ALU.bitwise_and)
                nc.vector.tensor_scalar(Ew[:], Ew[:], ALPHA, BETA, ALU.mult, ALU.add)
                nc.vector.tensor_tensor(Tlo[:], Ew[:], T36[:], ALU.subtract)
                nc.vector.scalar_tensor_tensor(Thi[:], Ew[:], -2.0, Tlo[:], ALU.mult, ALU.add)
                nc.vector.tensor_copy(B5H[0:4, :], B5[0:4, :])
                for dstt, srct in ((B5, Tlo), (B5H, Thi)):
                    tpp = psmq.tile([NT, 128], F32, tag="mq")
                    nc.tensor.transpose(tpp[:], srct[:], identt[:])
                    tst = sm.tile([NT, 128], F32, tag="tst")
                    nc.vector.tensor_copy(tst[:], tpp[:])
                    nc.sync.dma_start(dstt[4:5, 0:RP], tst[:])

                if it == 0:
                    nc.sync.dma_start(dbg[:, 0:NT], T36[:])
                # ---------- column phase ----------
                last = it == n_iters - 1
                for ic in range(0) if skip_col else range(len(IC_W)):
                    icw, ico = IC_W[ic], IC_OFF[ic]
                    nq = icw // 512
                    psA = psmq.tile([4, 1024], F32, tag="mq")   # sign-map accum
                    psB = psmq.tile([4, 1024], F32, tag="mq")   # gt-map accum
                    for J in range(JT):
                        jsl = slice(J * 128, (J + 1) * 128)
                        zl = zcm[:, J * 4:J * 4 + 4]
                        ps = psbig.tile([128, 1024], F32, tag="ps")
                        for q in range(nq):
                            isl = slice(ico + q * 512, ico + (q + 1) * 512)
                            nc.tensor.matmul(ps[:, q * 512:(q + 1) * 512],
                                             A5[0:5, jsl], B5[0:5, isl],
                                             start=True, stop=True)
                        Ms = msk.tile([128, 1024], F32, tag="Ms")
                        nc.scalar.activation(Ms[:, 0:icw], ps[:, 0:icw], AF.Sign)
                        for q in range(nq):
                            isl = slice(ico + q * 512, ico + (q + 1) * 512)
                            nc.tensor.matmul(ps[:, q * 512:(q + 1) * 512],
                                             A5[0:5, jsl], B5H[0:5, isl],
                                             start=True, stop=True)
                        Mb = msk.tile([128, 1024], F32, tag="Mb")
                        nc.vector.tensor_scalar(Mb[:, 0:icw], ps[:, 0:icw], 0.0, None, ALU.is_gt)
                        for q in range(nq):
                            qsl = slice(q * 512, (q + 1) * 512)
                            nc.tensor.matmul(psA[:, qsl], zl, Ms[:, qsl],
                                             start=(J == 0), stop=(J == JT - 1),
                                             skip_group_check=True)
                            nc.tensor.matmul(psB[:, qsl], zl, Mb[:, qsl],
                                             start=(J == 0), stop=(J == JT - 1),
                                             skip_group_check=True)
                    for q in range(nq):
                        qsl = slice(q * 512, (q + 1) * 512)
                        iso = ico + q * 512
                        cpA = epi.tile([4, 512], F32, tag="cpA")
                        cpB = epi.tile([4, 512], F32, tag="cpB")
                        nc.scalar.activation(cpA[:], psA[:, qsl], AF.Copy)
                        nc.vector.tensor_copy(cpB[:], psB[:, qsl])
                        cntA = epi.tile([1, 512], F32, tag="cntA")
                        cntB = epi.tile([1, 512], F32, tag="cntB")
                        nc.sync.dma_start(cntA[:], cpA[3:4, :])
                        nc.sync.dma_start(cntB[:], cpB[3:4, :])
                        if it == 0 and ic == 0 and q == 0:
                            nc.sync.dma_start(dbg2[0:4, :], cpA[:])
                            nc.sync.dma_start(dbg2[4:8, :], cpB[:])
                        # cpA[0:3] <- Sum_A' = (P_s + g)/2 ; then <- Dz = Sum_A' - Sum_B'
                        nc.vector.tensor_scalar(cpA[0:3, :], cpA[0:3, :], gs3, 0.5,
                                                ALU.add, ALU.mult)
                        nc.vector.tensor_tensor(cpA[0:3, :], cpA[0:3, :], cpB[0:3, :],
                                                ALU.subtract)
                        # wnum = (16 - nB)/16 ; den = max((C_s+HW)/2 - C_b, 0.5)
                        wnum = epi.tile([1, 512], F32, tag="wnum")
                        nc.vector.tensor_scalar(wnum[:], cntB[:], -1.0 / 16.0, 1.0,
                                                ALU.mult, ALU.add)
                        nc.vector.tensor_scalar(cntA[:], cntA[:], float(HW), 0.5,
                                                ALU.add, ALU.mult)
                        nc.vector.tensor_tensor(cntA[:], cntA[:], cntB[:], ALU.subtract)
                        nc.vector.tensor_scalar(cntA[:], cntA[:], 0.5, None, ALU.max)
                        nc.vector.reciprocal(cntA[:], cntA[:])
                        nc.vector.tensor_tensor(wnum[:], wnum[:], cntA[:], ALU.mult)
                        wrep = psbig.tile([3, 1024], F32, tag="ps")
                        nc.tensor.matmul(wrep[:, 0:512], o13[:], wnum[:], start=True, stop=True)
                        # cpA[0:3] <- (w/16)*Dz ; cpB[0:3] <- m = cpB/16 + that
                        nc.vector.tensor_tensor(cpA[0:3, :], wrep[:, 0:512], cpA[0:3, :],
                                                ALU.mult)
                        nc.vector.scalar_tensor_tensor(cpB[0:3, :], cpB[0:3, :], 1.0 / 16.0,
                                                       cpA[0:3, :], ALU.mult, ALU.add)
                        H6 = epi.tile([6, 512], F32, tag="H6")
                        if it == 0:
                            nc.sync.dma_start(H6[0:3, :], h0o[:, iso:iso + 512])
                        else:
                            nc.vector.tensor_copy(H6[0:3, :], B5[0:3, iso:iso + 512])
                        nc.sync.dma_start(H6[3:6, :], cpB[0:3, :])
                        qps = psbig.tile([3, 1024], F32, tag="ps")
                        nc.tensor.matmul(qps[:, 0:512], qwt[:], H6[:], start=True, stop=True)
                        hn = chk.tile([3, 512], F32, tag="hn")
                        nc.scalar.activation(hn[:], qps[:, 0:512], AF.Relu, bias=cw[:, 12:13])
                        if last:
                            nc.sync.dma_start(out[:, iso:iso + 512], hn[:])
                        else:
                            nc.sync.dma_start(ccin[:, iso:iso + 512], hn[:])

                if not last:
                    nc.gpsimd.collective_compute(
                        "AllGather", ALU.bypass,
                        replica_groups=[[0, 1], [2, 3], [4, 5], [6, 7]],
                        ins=[ccin.opt()], outs=[ccout.opt()])
                    nc.sync.dma_start(hT[:, 0:RP], ccout[0:3, :])
                    nc.sync.dma_start(hT[:, RP:], ccout[3:6, :])

    nc.compile()
    return nc


_CACHE = {}


def _get_program():
    if "nc" not in _CACHE:
        _CACHE["nc"] = _build_program()
    return _CACHE["nc"]


def kernel(cnn_encoder_output, proj_3d, g_W, g_b, g_a, q_W, q_b,
           gnn_iterations, k, **_unused):
    assert int(gnn_iterations) == 3 and int(k) == 16
    cnn = np.ascontiguousarray(np.asarray(cnn_encoder_output, np.float32))
    proj = np.ascontiguousarray(np.asarray(proj_3d, np.float32))
    g_W = np.asarray(g_W, np.float32)
    g_b = np.asarray(g_b, np.float32)
    g_a = np.asarray(g_a, np.float32)
    q_W = np.asarray(q_W, np.float32)
    q_b = np.asarray(q_b, np.float32)

    gw = np.ascontiguousarray(np.concatenate([g_W[0].T, g_W[1].T], axis=1), np.float32)
    gb = np.ascontiguousarray(np.stack([g_b[0], g_b[1]], axis=1), np.float32)
    ga = np.ascontiguousarray(np.broadcast_to(np.asarray(g_a)[None, :], (3, 2)), np.float32)
    qw = np.ascontiguousarray(q_W.T, np.float32)
    qbv = np.ascontiguousarray(q_b.reshape(3, 1), np.float32)
    ident = np.eye(128, dtype=np.float32)
    ones3 = np.ones((3, 1), np.float32)
    onesrow = np.ones((1, HW), np.float32)

    nc = _get_program()
    in_maps = []
    for core in range(8):
        b, half = core // 2, core % 2
        m01v = np.zeros((3, 2), np.float32)
        m01v[:, half] = 1.0
        xf = np.ascontiguousarray(cnn[b].reshape(3, HW))
        in_maps.append({
            "x0": xf,
            "pj": np.ascontiguousarray(proj[b].T),
            "h0own": np.ascontiguousarray(xf[:, half * RP:(half + 1) * RP]),
            "ident": ident,
            "m01": m01v,
            "gw": gw, "gb": gb, "ga": ga, "qw": qw, "qb": qbv,
            "ones3": ones3, "onesrow": onesrow, "zrow": np.zeros((1, RP), np.float32),
        })

    res = run_bass_kernel_spmd(nc, in_maps, list(range(8)),
                               trace=bool(int(os.environ.get("KBTRACE", "0"))))
    outs = res.results
    _CACHE["raw"] = outs
    _CACHE["exec_ns"] = res.exec_time_ns
    full = np.zeros((N, 3, HW), np.float32)
    for core in range(8):
        b, half = core // 2, core % 2
        full[b, :, half * RP:(half + 1) * RP] = outs[core]["out"]
    return full.reshape(N, 3, H, W)

